# revision 9
# baseline (speedup 1.0000x reference)
"""MADESplit forward on 8 TRN2 NeuronCores.

Strategy:
- Data-parallel: batch 8192 -> 8 shards of 1024 rows; weights replicated.
- Masks are constants (shape-derived): fold them into the weights on host.
- Degree-sort permutation of the 2048 hidden units makes the masked W1
  block-lower-triangular at 128x128 tile granularity -> skip all-zero tiles
  (~47% of the dominant GEMM's FLOPs and bytes).
- Feature-major activation layout [feature, batch] on chip; x is transposed
  on the PE array once, and u transposed back at the end.
- All GEMMs in float32r (PE runs it 4x faster than fp32; ~1.8e-4 rounding).
"""
import sys
sys.path.insert(0, '/opt/trn_rl_repo')
import numpy as np

import concourse.bacc as bacc
import concourse.mybir as mybir
import concourse.tile as tile
from concourse.bass_utils import run_bass_kernel_spmd

F32 = mybir.dt.float32
F32R = mybir.dt.float32r
AF = mybir.ActivationFunctionType

B, D, H = 8192, 128, 2048
NCORES = 8
BL = B // NCORES          # 1024 rows per core
NCH = 2                   # batch chunks per core
NC = BL // NCH            # 512 = matmul moving dim
FT = H // 128             # 16 feature tiles
BT = BL // 128            # 8 batch tiles of 128


def _masks():
    hd = np.arange(H) % (D - 1)
    m_in = (hd[:, None] >= np.arange(D)[None, :]).astype(np.float32)          # [H, D]
    m_h = (hd[:, None] >= hd[None, :]).astype(np.float32)                     # [H, H]
    m_out = ((np.arange(D) - 1)[:, None] >= hd[None, :]).astype(np.float32)   # [D, H]
    perm = np.argsort(hd, kind='stable')
    return m_in, m_h, m_out, perm


_M_IN, _M_H, _M_OUT, _PERM = _masks()

# nonzero 128x128 tiles of W1'.T (rows f', cols g'): any(mask_p[g' in gJ, f' in kI])
_MASK_P = _M_H[_PERM][:, _PERM]
_NZK = []  # per gJ: list of kI with a nonzero tile
for gJ in range(FT):
    blk = _MASK_P[gJ * 128:(gJ + 1) * 128]
    _NZK.append([kI for kI in range(FT)
                 if blk[:, kI * 128:(kI + 1) * 128].any()])
_NNZ_TOT = sum(len(k) for k in _NZK)


def _prep_mlp(w0, b0, w1, b1, w2, b2):
    """Host-side: fold masks, permute, build packed lhsT arrays (fp32)."""
    p = _PERM
    w0p = (_M_IN * w0)[p, :]                  # [H, D]
    w1p = (_M_H * w1)[p][:, p]                # [H, H]
    w2p = (_M_OUT * w2)[:, p]                 # [D, H]
    w0T = np.ascontiguousarray(w0p.T)         # [D=128, H] lhsT for L0
    w1T = np.ascontiguousarray(w1p.T)         # [f, g]
    blocks = []
    for gJ in range(FT):
        for kI in _NZK[gJ]:
            blocks.append(w1T[kI * 128:(kI + 1) * 128, gJ * 128:(gJ + 1) * 128])
    w1pack = np.ascontiguousarray(np.concatenate(blocks, axis=1))  # [128, 128*nnz]
    w2T = np.ascontiguousarray(w2p.T.reshape(FT, 128, D).transpose(1, 0, 2)
                               .reshape(128, FT * D))              # [128, 16*128]
    b0p = np.ascontiguousarray(b0[p].reshape(FT, 128).T)           # [128, 16]
    b1p = np.ascontiguousarray(b1[p].reshape(FT, 128).T)           # [128, 16]
    b2c = np.ascontiguousarray(b2.reshape(D, 1))                   # [128, 1]
    return (w0T.astype(np.float32), w1pack.astype(np.float32),
            w2T.astype(np.float32), b0p.astype(np.float32),
            b1p.astype(np.float32), b2c.astype(np.float32))


_NC_CACHE = None


def _build():
    nc = bacc.Bacc("TRN2", target_bir_lowering=False, debug=False)
    x_d = nc.dram_tensor("x", [BL, D], F32, kind="ExternalInput")
    ones_d = nc.dram_tensor("ones", [128, 1], F32R, kind="ExternalInput")
    wd = {}
    for m in ("s", "t"):
        wd[m] = dict(
            w0=nc.dram_tensor(f"{m}_w0T", [128, H], F32R, kind="ExternalInput"),
            w1=nc.dram_tensor(f"{m}_w1p", [128, 128 * _NNZ_TOT], F32R,
                              kind="ExternalInput"),
            w2=nc.dram_tensor(f"{m}_w2T", [128, FT * D], F32R,
                              kind="ExternalInput"),
            b0=nc.dram_tensor(f"{m}_b0", [128, FT], F32, kind="ExternalInput"),
            b1=nc.dram_tensor(f"{m}_b1", [128, FT], F32, kind="ExternalInput"),
            b2=nc.dram_tensor(f"{m}_b2", [128, 1], F32, kind="ExternalInput"),
        )
    u_d = nc.dram_tensor("u", [BL, D], F32, kind="ExternalOutput")
    ld_d = nc.dram_tensor("ld", [1, BL], F32, kind="ExternalOutput")

    with tile.TileContext(nc) as tc:
        with (
            tc.tile_pool(name="wgt", bufs=1) as wgt,      # big weight tiles
            tc.tile_pool(name="act", bufs=1) as actp,     # activations
            tc.tile_pool(name="sm", bufs=2) as sm,        # small tiles
            tc.tile_pool(name="ph0", bufs=2, space="PSUM") as ph0,
            tc.tile_pool(name="ph1", bufs=2, space="PSUM") as ph1,
            tc.tile_pool(name="pl2", bufs=1, space="PSUM") as pl2,
            tc.tile_pool(name="ptr", bufs=2, space="PSUM") as ptr,
            tc.tile_pool(name="pld", bufs=1, space="PSUM") as pld,
        ):
            ident_np = np.eye(128, dtype=np.float32)
            ident_d = nc.inline_tensor(ident_np, name="ident_c")
            ident = sm.tile([128, 128], F32, tag="ident")
            nc.sync.dma_start(ident[:], ident_d[:])
            ones_t = sm.tile([128, 1], F32R, tag="ones")
            nc.sync.dma_start(ones_t[:], ones_d[:])

            # ---- transpose x into feature-major xT (f32 and f32r copies),
            # one tile per batch chunk so chunk deps stay independent
            xT32 = [actp.tile([128, NC], F32, tag=f"xT32_{c}", name=f"xT32_{c}") for c in range(NCH)]
            xTr = [actp.tile([128, NC], F32R, tag=f"xTr_{c}", name=f"xTr_{c}") for c in range(NCH)]
            for bI in range(BT):
                xin = sm.tile([128, D], F32, tag="xin")
                nc.sync.dma_start(xin[:], x_d[bI * 128:(bI + 1) * 128, :])
                xps = ptr.tile([128, 128], F32, tag="tr")
                nc.tensor.transpose(xps[:], xin[:], ident[:])
                c, col = bI // (NC // 128), (bI % (NC // 128)) * 128
                nc.vector.tensor_copy(xT32[c][:, col:col + 128], xps[:])
                nc.scalar.activation(xTr[c][:, col:col + 128], xps[:], AF.Identity)

            mT = [actp.tile([128, NC], F32, tag=f"mT_{c}", name=f"mT_{c}") for c in range(NCH)]
            aT = [actp.tile([128, NC], F32R, tag=f"aT_{c}", name=f"aT_{c}") for c in range(NCH)]

            for m, actf in (("s", AF.Tanh), ("t", AF.Relu)):
                w = wd[m]
                w0_t = wgt.tile([128, H], F32R, tag="w0")
                nc.sync.dma_start(w0_t[:], w["w0"][:])
                b0_t = sm.tile([128, FT], F32, tag="b0")
                nc.sync.dma_start(b0_t[:], w["b0"][:])
                b1_t = sm.tile([128, FT], F32, tag="b1")
                nc.sync.dma_start(b1_t[:], w["b1"][:])
                b2_t = sm.tile([128, 1], F32, tag="b2")
                nc.sync.dma_start(b2_t[:], w["b2"][:])
                # w1 packed blocks: one tile per 2-g-group piece so L1 for
                # the first g-tiles starts as soon as its own piece lands,
                # and the next MLP's piece loads overlap this MLP's tail.
                GPP = 2  # g-tiles per piece
                piece_tiles, piece_off = [], []
                off_b = 0
                for p in range(FT // GPP):
                    n_ = sum(len(_NZK[g]) for g in range(p * GPP, (p + 1) * GPP))
                    pt = wgt.tile([128, 128 * n_], F32R, tag=f"w1p{p}")
                    nc.sync.dma_start(
                        pt[:], w["w1"][:, off_b * 128:(off_b + n_) * 128])
                    piece_tiles.append(pt)
                    piece_off.append(off_b)
                    off_b += n_
                w2_t = wgt.tile([128, FT * D], F32R, tag="w2")
                nc.sync.dma_start(w2_t[:], w["w2"][:])

                for c in range(NCH):
                    csl = slice(c * NC, (c + 1) * NC)
                    # ---- L0: h0T[fI] = act(w0T[:,fI].T @ xTr + b0)
                    h0 = []
                    for fI in range(FT):
                        ps = ph0.tile([128, NC], F32, tag="h0ps")
                        nc.tensor.matmul(ps[:], w0_t[:, fI * 128:(fI + 1) * 128],
                                         xTr[c][:], start=True, stop=True)
                        ht = actp.tile([128, NC], F32R, tag=f"h0_{fI}")
                        nc.scalar.activation(ht[:], ps[:], actf,
                                             bias=b0_t[:, fI:fI + 1])
                        h0.append(ht)
                    # ---- L1: h1T[gJ] = act(sum_k w1T[k,g].T @ h0[k] + b1)
                    h1 = []
                    off = 0
                    for gJ in range(FT):
                        ks = _NZK[gJ]
                        ps = ph1.tile([128, NC], F32, tag="h1ps")
                        w1_t = piece_tiles[gJ // GPP]
                        woff = piece_off[gJ // GPP]
                        for j, kI in enumerate(ks):
                            wsl = slice((off + j - woff) * 128,
                                        (off + j + 1 - woff) * 128)
                            nc.tensor.matmul(ps[:], w1_t[:, wsl], h0[kI][:],
                                             start=(j == 0),
                                             stop=(j == len(ks) - 1))
                        off += len(ks)
                        ht = actp.tile([128, NC], F32R, tag=f"h1_{gJ}")
                        nc.scalar.activation(ht[:], ps[:], actf,
                                             bias=b1_t[:, gJ:gJ + 1])
                        h1.append(ht)
                    # ---- L2: out = sum_k w2T[k].T @ h1[k] + b2
                    ps = pl2.tile([128, NC], F32, tag="l2ps")
                    for kI in range(FT):
                        nc.tensor.matmul(ps[:], w2_t[:, kI * 128:(kI + 1) * 128],
                                         h1[kI][:], start=(kI == 0),
                                         stop=(kI == FT - 1))
                    if m == "s":
                        nc.scalar.activation(mT[c][:], ps[:], AF.Identity,
                                             bias=b2_t[:, 0:1])
                    else:
                        nc.scalar.activation(aT[c][:], ps[:], AF.Identity,
                                             bias=b2_t[:, 0:1])

            # ---- final: u = (x - m) * exp(-a); ld = -sum_o a
            for c in range(NCH):
                csl = slice(c * NC, (c + 1) * NC)
                ex = actp.tile([128, NC], F32, tag=f"ex_{c}")
                nc.scalar.activation(ex[:], aT[c][:].bitcast(F32), AF.Exp, scale=-1.0)
                df = actp.tile([128, NC], F32, tag=f"df_{c}")
                nc.vector.tensor_sub(df[:], xT32[c][:], mT[c][:])
                uT = actp.tile([128, NC], F32, tag=f"uT_{c}")
                nc.vector.tensor_mul(uT[:], df[:], ex[:])
                for bI in range(NC // 128):
                    ups = ptr.tile([128, 128], F32, tag="tr")
                    nc.tensor.transpose(ups[:], uT[:, bI * 128:(bI + 1) * 128],
                                        ident[:])
                    usb = sm.tile([128, D], F32, tag="usb")
                    nc.vector.tensor_copy(usb[:], ups[:])
                    r0 = c * NC + bI * 128
                    nc.sync.dma_start(u_d[r0:r0 + 128, :], usb[:])
                lps = pld.tile([1, NC], F32, tag="ldps")
                nc.tensor.matmul(lps[:], ones_t[:], aT[c][:],
                                 start=True, stop=True)
                lsb = sm.tile([1, NC], F32, tag="ldsb")
                nc.scalar.activation(lsb[:], lps[:], AF.Identity, scale=-1.0)
                nc.sync.dma_start(ld_d[0:1, csl], lsb[:])

    nc.compile()
    return nc


def kernel(inputs, s_w0, s_b0, s_w1, s_b1, s_w2, s_b2,
           t_w0, t_b0, t_w1, t_b1, t_w2, t_b2, _trace=False):
    global _NC_CACHE
    if _NC_CACHE is None:
        _NC_CACHE = _build()
    nc = _NC_CACHE

    f = np.asarray
    sp = _prep_mlp(f(s_w0), f(s_b0), f(s_w1), f(s_b1), f(s_w2), f(s_b2))
    tp = _prep_mlp(f(t_w0), f(t_b0), f(t_w1), f(t_b1), f(t_w2), f(t_b2))
    ones = np.ones((128, 1), np.float32)
    x = np.ascontiguousarray(np.asarray(inputs, dtype=np.float32))

    base = {"ones": ones}
    for m, p in (("s", sp), ("t", tp)):
        w0T, w1pack, w2T, b0p, b1p, b2c = p
        base.update({f"{m}_w0T": w0T, f"{m}_w1p": w1pack, f"{m}_w2T": w2T,
                     f"{m}_b0": b0p, f"{m}_b1": b1p, f"{m}_b2": b2c})
    in_maps = [dict(base, x=x[i * BL:(i + 1) * BL]) for i in range(NCORES)]

    res = run_bass_kernel_spmd(nc, in_maps, list(range(NCORES)), trace=_trace)
    u = np.concatenate([r["u"] for r in res.results], axis=0)
    ld = np.concatenate([r["ld"].reshape(BL, 1) for r in res.results], axis=0)
    if _trace:
        kernel._last_exec_ns = res.exec_time_ns
    return u, ld
